# revision 1
# baseline (speedup 1.0000x reference)
"""LocalGraphMessageBlock TRN2 kernel.

Math (per chunk of C=512 tokens, H=256 features, offsets 1,2,4,8):
  h_in = LN(h);  per offset o and direction:
    z = P[dst] + Q[src] + C5 @ e  (+b1 folded into P, -BIG*(1-mask) folded
        into the K=20 e-matmul so gelu(z)=0 exactly on invalid edges)
    agg += gelu(z) @ w2           (PSUM accumulation across all 8 dirs)
  agg += b2 (x) deg               (outer product, deg = #valid edges per dst)
  h2 = h + agg;  out = (h2 + MLP(LN(h2))) * valid

Layout: feature-major on chip ([128 feat, 512 tok] tiles); tokens enter/leave
token-major via PE transposes. P = A^T h_in, Q = B^T h_in precomputed per
chunk (A,B = halves of w1). rsqrt computed on DVE via bit-hack + 3 Newton
steps on transpose-packed [128, 4r] tiles (keeps ACT on the gelu table set
for the whole kernel).

Data-parallel over the chunk dim N: 256 chunks / 8 cores = 32 chunks each,
same NEFF, per-core input slices.
"""
import json

import numpy as np

N_TOT, C, H = 256, 512, 256
OFFSETS = (1, 2, 4, 8)
N_CORES = 8
CPC = N_TOT // N_CORES  # chunks per core
BIG = 30000.0
EPS = 1e-5
MAGIC = 0x5F3759DF

# ---------------------------------------------------------------------------
# Walrus workaround: this container's walrus accepts at most ONE sync-wait
# command per instruction; Tile emits more. Split excess onto preceding
# NoOps on the same engine (engine queues are in-order, so this is
# equivalent gating).
# ---------------------------------------------------------------------------
_patched = False


def _split_sync_waits(bir_json: bytes, maxw: int = 1) -> bytes:
    m = json.loads(bir_json)
    cnt = 0
    changed = False
    for f in m.get("functions", []):
        for blk in f.get("blocks", []):
            newins = []
            for ins in blk.get("instructions", []):
                si = ins.get("sync_info")
                if si:
                    waits = si.get("on_wait") or []
                    if len(waits) > maxw:
                        changed = True
                        si["on_wait"] = waits[-maxw:]
                        extra = waits[:-maxw]
                        for i in range(0, len(extra), maxw):
                            cnt += 1
                            newins.append({
                                "debug": ins.get("debug", 0),
                                "engine": ins["engine"],
                                "ins": [], "outs": [],
                                "name": f"{ins['name']}-ws{cnt}",
                                "opcode": "NoOp",
                                "sync_info": {"on_update": [],
                                              "on_wait": extra[i:i + maxw]},
                            })
                newins.append(ins)
            blk["instructions"] = newins
    return json.dumps(m).encode() if changed else bir_json


def _install_patch():
    global _patched
    if _patched:
        return
    import concourse.bass_utils as bu
    import concourse.bass2jax as b2j

    orig = bu.compile_bir_kernel

    def patched(bir_json, tmpdir, neff_name="file.neff"):
        return orig(_split_sync_waits(bir_json), tmpdir, neff_name)

    bu.compile_bir_kernel = patched
    b2j.compile_bir_kernel = patched
    _patched = True


# ---------------------------------------------------------------------------
# Bass kernel builder
# ---------------------------------------------------------------------------
_nc_cache = {}


def _build(n_chunks):
    import concourse.bass as bass
    import concourse.tile as tile
    from concourse import mybir

    f32 = mybir.dt.float32
    i32 = mybir.dt.int32
    u8 = mybir.dt.uint8
    AF = mybir.ActivationFunctionType
    ALU = mybir.AluOpType

    nc = bass.Bass("TRN2")

    # ---- dram I/O ----
    h_d = nc.dram_tensor("h", [n_chunks, C, H], f32, kind="ExternalInput")
    xyz_d = nc.dram_tensor("xyz", [n_chunks, C, 3], f32, kind="ExternalInput")
    val_d = nc.dram_tensor("valid", [n_chunks, C], u8, kind="ExternalInput")
    out_d = nc.dram_tensor("out", [n_chunks, C, H], f32, kind="ExternalOutput")

    def din(name, shape, dt=f32):
        return nc.dram_tensor(name, shape, dt, kind="ExternalInput")

    A_d = din("A", [2, 128, 512])       # w1[:256] k-halves  (lhsT blocks)
    B_d = din("B", [2, 128, 512])       # w1[256:512]
    W2_d = din("W2", [128, 1024])       # w2 k-blocks: [:, k*256:(k+1)*256]
    U1_d = din("U1", [2, 128, 1024])
    U2_d = din("U2", [128, 2048])       # u2 k-blocks: [:, k*256:(k+1)*256]
    CF_d = din("CF", [20, 4096])        # e-matmul lhsT blocks (oi,dir,m)
    b1c_d = din("b1c", [128, 4])
    bu1c_d = din("bu1c", [128, 8])
    bu2c_d = din("bu2c", [128, 2])
    b2r_d = din("b2r", [1, 256])
    lnw_d = din("lnw", [1, 256])
    lnb_d = din("lnb", [1, 256])
    lnuw_d = din("lnuw", [1, 256])
    lnub_d = din("lnub", [1, 256])
    onesH_d = din("onesH", [128, 1])    # 1/H
    ones4_d = din("ones4", [4, 1])
    seldsq_d = din("seldsq", [12, 4])
    selbc_d = din("selbc", [4, 12])
    onesr_d = din("onesr", [1, 512])
    ident_d = din("ident", [128, 128])
    seed_d = din("seed", [128, 24], i32)

    from contextlib import ExitStack
    with tile.TileContext(nc) as tc, ExitStack() as ctx:
        cp = ctx.enter_context(tc.tile_pool(name="consts", bufs=1))
        ld = ctx.enter_context(tc.tile_pool(name="loads", bufs=2))
        wk = ctx.enter_context(tc.tile_pool(name="work", bufs=2))
        sm = ctx.enter_context(tc.tile_pool(name="small", bufs=1))
        pmm = ctx.enter_context(tc.tile_pool(name="pmm", bufs=3, space="PSUM"))
        pagg = ctx.enter_context(tc.tile_pool(name="pagg", bufs=1, space="PSUM"))
        pms = ctx.enter_context(tc.tile_pool(name="pms", bufs=3, space="PSUM"))

        # ---- load constants ----
        def cload(dram, shape, dt=f32, name=None):
            t = cp.tile(shape, dt, name=name, tag=name)
            nc.sync.dma_start(t, dram[tuple(slice(None) for _ in shape)])
            return t

        A0 = cp.tile([128, 512], f32); nc.sync.dma_start(A0, A_d[0])
        A1 = cp.tile([128, 512], f32); nc.sync.dma_start(A1, A_d[1])
        B0 = cp.tile([128, 512], f32); nc.sync.dma_start(B0, B_d[0])
        B1 = cp.tile([128, 512], f32); nc.sync.dma_start(B1, B_d[1])
        W2 = cload(W2_d, [128, 1024], name="W2")
        U1a = cp.tile([128, 1024], f32); nc.sync.dma_start(U1a, U1_d[0])
        U1b = cp.tile([128, 1024], f32); nc.sync.dma_start(U1b, U1_d[1])
        U2 = cload(U2_d, [128, 2048], name="U2")
        CF = cload(CF_d, [20, 4096], name="CF")
        b1c = cload(b1c_d, [128, 4], name="b1c")
        bu1c = cload(bu1c_d, [128, 8], name="bu1c")
        bu2c = cload(bu2c_d, [128, 2], name="bu2c")
        b2r = cload(b2r_d, [1, 256], name="b2r")
        lnw = cload(lnw_d, [1, 256], name="lnw")
        lnb = cload(lnb_d, [1, 256], name="lnb")
        lnuw = cload(lnuw_d, [1, 256], name="lnuw")
        lnub = cload(lnub_d, [1, 256], name="lnub")
        onesH = cload(onesH_d, [128, 1], name="onesH")
        ones4 = cload(ones4_d, [4, 1], name="ones4")
        seldsq = cload(seldsq_d, [12, 4], name="seldsq")
        selbc = cload(selbc_d, [4, 12], name="selbc")
        onesr = cload(onesr_d, [1, 512], name="onesr")
        ident = cload(ident_d, [128, 128], name="ident")
        seed = cload(seed_d, [128, 24], i32, name="seed")

        def rsqrt_rows(rows, r, tag):
            """rows: [r, 512] sbuf (positive) -> [r, 512] sbuf of 1/sqrt."""
            w = 4 * r
            rt = pms.tile([128, w], f32, name="rt", tag="miscps", bufs=2)
            for g in range(4):
                nc.tensor.transpose(rt[:, g * r:(g + 1) * r],
                                    rows[:, g * 128:(g + 1) * 128],
                                    ident[0:r, 0:r])
            x = sm.tile([128, w], f32, name=f"nrx{tag}", tag=f"nrx{tag}")
            nc.vector.tensor_copy(x, rt)
            yi = sm.tile([128, w], i32, name=f"nry{tag}", tag=f"nry{tag}")
            nc.vector.tensor_scalar(out=yi, in0=x.bitcast(i32), scalar1=1,
                                    scalar2=None, op0=ALU.logical_shift_right)
            nc.vector.tensor_sub(yi, seed[:, 0:w], yi)
            y = yi.bitcast(f32)
            t = sm.tile([128, w], f32, name=f"nrt{tag}", tag=f"nrt{tag}")
            for _ in range(3):
                nc.vector.tensor_mul(t, y, y)
                nc.vector.tensor_mul(t, t, x)
                nc.vector.tensor_scalar(out=t, in0=t, scalar1=-0.5,
                                        scalar2=1.5, op0=ALU.mult, op1=ALU.add)
                nc.vector.tensor_mul(y, y, t)
            rp = pms.tile([r, 512], f32, name="rp", tag="miscps", bufs=2)
            for g in range(4):
                nc.tensor.transpose(rp[:, g * 128:(g + 1) * 128],
                                    y[:, g * r:(g + 1) * r], ident)
            out = sm.tile([r, 512], f32, name=f"nro{tag}", tag=f"nro{tag}")
            nc.vector.tensor_copy(out, rp)
            return out

        def ln_fm(hfm, wrow, brow, tag):
            """Feature-major layernorm of hfm (2x [128,512]) -> 2 tiles."""
            mu_ps = pms.tile([1, 512], f32, name="mu_ps", tag="miscps", bufs=2)
            m2_ps = pms.tile([1, 512], f32, name="m2_ps", tag="miscps", bufs=2)
            for fh in range(2):
                x2 = wk.tile([128, 512], f32, name=f"x2{tag}", tag=f"x2{tag}", bufs=1)
                nc.gpsimd.tensor_mul(x2, hfm[fh], hfm[fh])
                nc.tensor.matmul(mu_ps, onesH, hfm[fh], start=(fh == 0),
                                 stop=(fh == 1))
                nc.tensor.matmul(m2_ps, onesH, x2, start=(fh == 0),
                                 stop=(fh == 1))
            mu_row = sm.tile([1, 512], f32, name=f"mur{tag}", tag=f"mur{tag}")
            nc.scalar.copy(mu_row, mu_ps)
            mumu = sm.tile([1, 512], f32, name=f"mumu{tag}", tag=f"mumu{tag}")
            nc.scalar.activation(mumu, mu_ps, AF.Square)
            vare = sm.tile([1, 512], f32, name=f"var{tag}", tag=f"var{tag}")
            nc.vector.scalar_tensor_tensor(out=vare, in0=m2_ps, scalar=EPS,
                                           in1=mumu, op0=ALU.add,
                                           op1=ALU.subtract)
            rstd = rsqrt_rows(vare, 1, tag)
            sh_row = sm.tile([1, 512], f32, name=f"shr{tag}", tag=f"shr{tag}")
            nc.vector.scalar_tensor_tensor(out=sh_row, in0=mu_row, scalar=-1.0,
                                           in1=rstd, op0=ALU.mult,
                                           op1=ALU.mult)
            outs = []
            for fh in range(2):
                arep = pms.tile([128, 512], f32, name="arep", tag="miscps", bufs=2)
                nc.tensor.matmul(arep, wrow[:, fh * 128:(fh + 1) * 128], rstd,
                                 start=True, stop=True)
                brep = pms.tile([128, 512], f32, name="brep", tag="miscps", bufs=2)
                nc.tensor.matmul(brep, wrow[:, fh * 128:(fh + 1) * 128],
                                 sh_row, start=True, stop=False)
                nc.tensor.matmul(brep, brow[:, fh * 128:(fh + 1) * 128],
                                 onesr, start=False, stop=True)
                o = wk.tile([128, 512], f32, name=f"ln{tag}{fh}", tag=f"ln{tag}{fh}", bufs=1)
                nc.vector.tensor_mul(o, hfm[fh], arep)
                nc.vector.tensor_add(o, o, brep)
                outs.append(o)
            return outs

        for ci in range(n_chunks):
            # ---------------- loads ----------------
            ht = ld.tile([128, 1024], f32, name="ht", tag="ht")
            hb = h_d[ci]
            nc.sync.dma_start(
                ht.rearrange("p (i f) -> p i f", i=4),
                bass.AP(tensor=hb.tensor, offset=hb.offset,
                        ap=[[256, 128], [128 * 256, 4], [1, 256]]))
            xyzp = sm.tile([3, 520], f32, name="xyzp", tag="xyzp")
            nc.vector.memset(xyzp, 0.0)
            nc.sync.dma_start(xyzp[:, 0:512],
                              xyz_d[ci].rearrange("t k -> k t"))
            vbase = val_d[ci]
            vr_u8 = sm.tile([4, 512], u8, name="vru", tag="vru")
            nc.sync.dma_start(
                vr_u8, bass.AP(tensor=vbase.tensor, offset=vbase.offset,
                               ap=[[0, 4], [1, 512]]))
            vrf = sm.tile([4, 512], f32, name="vrf", tag="vrf")
            nc.gpsimd.tensor_copy(vrf, vr_u8)
            vrs_u8 = sm.tile([4, 512], u8, name="vrsu", tag="vrsu")
            nc.vector.memset(vrs_u8, 0)
            for oi, off in enumerate(OFFSETS):
                nc.sync.dma_start(vrs_u8[oi:oi + 1, 0:C - off],
                                  val_d[ci, off:C][None, :])
            vrsf = sm.tile([4, 512], f32, name="vrsf", tag="vrsf")
            nc.gpsimd.tensor_copy(vrsf, vrs_u8)
            vcol_u8 = sm.tile([128, 4], u8, name="vcu", tag="vcu")
            nc.sync.dma_start(
                vcol_u8, bass.AP(tensor=vbase.tensor, offset=vbase.offset,
                                 ap=[[1, 128], [128, 4]]))
            vcolf = sm.tile([128, 4], f32, name="vcf", tag="vcf")
            nc.vector.tensor_copy(vcolf, vcol_u8)

            # ---------------- h -> feature-major ----------------
            hfm = [wk.tile([128, 512], f32, name=f"hfm{fh}", tag=f"hfm{fh}") for fh in range(2)]
            for i in range(4):
                for fh in range(2):
                    tp = pms.tile([128, 128], f32, name="tp", tag="tpps", bufs=1)
                    nc.tensor.transpose(
                        tp, ht[:, i * 256 + fh * 128: i * 256 + (fh + 1) * 128],
                        ident)
                    nc.scalar.copy(hfm[fh][:, i * 128:(i + 1) * 128], tp)

            # ---------------- LN1 ----------------
            hin = ln_fm(hfm, lnw, lnb, "a")

            # ---------------- edge features ----------------
            delta = sm.tile([12, 512], f32, name="delta", tag="delta")
            for oi, off in enumerate(OFFSETS):
                dlo = sm.tile([3, 512], f32, name=f"dlo{oi}", tag=f"dlo{oi}")
                nc.vector.tensor_sub(dlo, xyzp[:, off:off + 512],
                                     xyzp[:, 0:512])
                nc.sync.dma_start(delta[3 * oi:3 * oi + 3], dlo)
            dsq = sm.tile([12, 512], f32, name="dsq", tag="dsq")
            nc.gpsimd.tensor_mul(dsq, delta, delta)
            d2_ps = pms.tile([4, 512], f32, name="d2_ps", tag="miscps", bufs=2)
            nc.tensor.matmul(d2_ps, seldsq, dsq, start=True, stop=True)
            R = sm.tile([4, 512], f32, name="Rrows", tag="Rrows")
            nc.vector.tensor_scalar(out=R[0:4], in0=d2_ps, scalar1=1e-12,
                                    scalar2=None, op0=ALU.max)

            m_all = sm.tile([4, 512], f32, name="mall", tag="mall")
            nc.gpsimd.tensor_mul(m_all, vrf, vrsf)
            e20 = sm.tile([20, 512], f32, name="e20", tag="e20")
            mm1 = sm.tile([4, 512], f32, name="mm1", tag="mm1")
            nc.vector.tensor_scalar(out=mm1, in0=m_all, scalar1=1.0,
                                    scalar2=None, op0=ALU.subtract)
            nc.sync.dma_start(e20[16:20], mm1)

            # LN1 stats row is produced inside ln_fm; R row 4 = var+eps of LN1
            # is handled there. Here R rows 0-3 (d2c) get their own rsqrt:
            rsq = rsqrt_rows(R, 4, "e")

            invrep = pms.tile([12, 512], f32, name="invrep", tag="miscps", bufs=2)
            nc.tensor.matmul(invrep, selbc, rsq[0:4], start=True, stop=True)
            nc.vector.tensor_mul(e20[0:12], delta, invrep)
            dst4 = sm.tile([4, 512], f32, name="dst4", tag="dst4")
            nc.vector.tensor_mul(dst4, R[0:4], rsq[0:4])
            nc.sync.dma_start(e20[12:16], dst4)

            # ---------------- degree / agg init ----------------
            mrev = sm.tile([4, 512], f32, name="mrev", tag="mrev")
            nc.vector.memset(mrev, 0.0)
            for oi, off in enumerate(OFFSETS):
                nc.sync.dma_start(mrev[oi:oi + 1, off:C],
                                  m_all[oi:oi + 1, 0:C - off])
            deg_ps = pms.tile([1, 512], f32, name="deg_ps", tag="miscps", bufs=2)
            nc.tensor.matmul(deg_ps, ones4, m_all, start=True, stop=False)
            nc.tensor.matmul(deg_ps, ones4, mrev, start=False, stop=True)
            deg_row = sm.tile([1, 512], f32, name="degr", tag="degr")
            nc.scalar.copy(deg_row, deg_ps)

            agg = [pagg.tile([128, 512], f32, name=f"agg{fh}", tag=f"agg{fh}")
                   for fh in range(2)]
            for fh in range(2):
                nc.tensor.matmul(agg[fh], b2r[:, fh * 128:(fh + 1) * 128],
                                 deg_row, start=True, stop=False,
                                 skip_group_check=True)

            # ---------------- P, Q ----------------
            P = wk.tile([128, 2048], f32, name="P", tag="P", bufs=1)
            Q = wk.tile([128, 2048], f32, name="Q", tag="Q", bufs=1)
            for m in range(4):
                pq = pmm.tile([128, 512], f32, name="mmps", tag="mmps")
                nc.tensor.matmul(pq, A0[:, m * 128:(m + 1) * 128], hin[0],
                                 start=True, stop=False)
                nc.tensor.matmul(pq, A1[:, m * 128:(m + 1) * 128], hin[1],
                                 start=False, stop=True)
                nc.scalar.activation(P[:, m * 512:(m + 1) * 512], pq,
                                     AF.Identity, bias=b1c[:, m:m + 1])
                pq2 = pmm.tile([128, 512], f32, name="mmps", tag="mmps")
                nc.tensor.matmul(pq2, B0[:, m * 128:(m + 1) * 128], hin[0],
                                 start=True, stop=False)
                nc.tensor.matmul(pq2, B1[:, m * 128:(m + 1) * 128], hin[1],
                                 start=False, stop=True)
                nc.scalar.copy(Q[:, m * 512:(m + 1) * 512], pq2)

            # ---------------- messages ----------------
            for oi, off in enumerate(OFFSETS):
                N = C - off
                for d in range(2):  # 0=fwd (dst=i), 1=rev (dst=j)
                    tzg = wk.tile([128, 2048], f32, name="tzg", tag="tzg")
                    po = 0 if d == 0 else off
                    qo = off if d == 0 else 0
                    gt = wk.tile([128, 2048], f32, name="gt", tag="gt",
                                 bufs=1)
                    for m in range(4):
                        blk = ((oi * 2 + d) * 4 + m) * 128
                        ze = pmm.tile([128, 512], f32, name="mmps", tag="mmps")
                        nc.tensor.matmul(ze[:, 0:N], CF[:, blk:blk + 128],
                                         e20[:, 0:N], start=True, stop=True)
                        sl = tzg[:, m * 512:m * 512 + N]
                        gl = gt[:, m * 512:m * 512 + N]
                        nc.gpsimd.tensor_add(
                            sl, P[:, m * 512 + po:m * 512 + po + N],
                            Q[:, m * 512 + qo:m * 512 + qo + N])
                        nc.vector.tensor_add(gl, sl, ze[:, 0:N])
                        nc.scalar.activation(sl, gl, AF.Gelu)
                    for fh in range(2):
                        dst = (agg[fh][:, 0:N] if d == 0
                               else agg[fh][:, off:off + N])
                        for k in range(4):
                            last = (oi == 3 and d == 1 and k == 3)
                            nc.tensor.matmul(
                                dst, W2[:, k * 256 + fh * 128:
                                        k * 256 + (fh + 1) * 128],
                                tzg[:, k * 512:k * 512 + N],
                                start=False, stop=last,
                                skip_group_check=True)

            # ---------------- update MLP ----------------
            h2 = [wk.tile([128, 512], f32, name=f"h2{fh}", tag=f"h2{fh}", bufs=1) for fh in range(2)]
            for fh in range(2):
                nc.vector.tensor_add(h2[fh], hfm[fh], agg[fh])
            hun = ln_fm(h2, lnuw, lnub, "u")
            gu = [wk.tile([128, 2048], f32, name=f"gu{g}", tag=f"gu{g}", bufs=1) for g in range(2)]
            for um in range(8):
                up = pmm.tile([128, 512], f32, name="mmps", tag="mmps")
                nc.tensor.matmul(up, U1a[:, um * 128:(um + 1) * 128], hun[0],
                                 start=True, stop=False)
                nc.tensor.matmul(up, U1b[:, um * 128:(um + 1) * 128], hun[1],
                                 start=False, stop=True)
                nc.scalar.activation(gu[um // 4][:, (um % 4) * 512:
                                                 (um % 4 + 1) * 512],
                                     up, AF.Gelu, bias=bu1c[:, um:um + 1])
            of = [wk.tile([128, 512], f32, name=f"of{fh}", tag=f"of{fh}", bufs=1) for fh in range(2)]
            for fh in range(2):
                u2p = pmm.tile([128, 512], f32, name="mmps", tag="mmps")
                for k in range(8):
                    nc.tensor.matmul(
                        u2p, U2[:, k * 256 + fh * 128:k * 256 + (fh + 1) * 128],
                        gu[k // 4][:, (k % 4) * 512:(k % 4 + 1) * 512],
                        start=(k == 0), stop=(k == 7))
                nc.vector.scalar_tensor_tensor(
                    out=of[fh], in0=u2p, scalar=bu2c[:, fh:fh + 1],
                    in1=h2[fh], op0=ALU.add, op1=ALU.add)

            # ---------------- transpose out + mask + store ----------------
            osb = ld.tile([128, 1024], f32, name="osb", tag="osb")
            for i in range(4):
                for fh in range(2):
                    tp = pms.tile([128, 128], f32, name="tp", tag="tpps", bufs=1)
                    nc.tensor.transpose(tp, of[fh][:, i * 128:(i + 1) * 128],
                                        ident)
                    nc.scalar.activation(
                        osb[:, i * 256 + fh * 128:i * 256 + (fh + 1) * 128],
                        tp, AF.Copy, bias=0.0, scale=vcolf[:, i:i + 1])
            ob = out_d[ci]
            nc.sync.dma_start(
                bass.AP(tensor=ob.tensor, offset=ob.offset,
                        ap=[[256, 128], [128 * 256, 4], [1, 256]]),
                osb.rearrange("p (i f) -> p i f", i=4))

    return nc


def _get_nc(n_chunks):
    if n_chunks not in _nc_cache:
        _nc_cache[n_chunks] = _build(n_chunks)
    return _nc_cache[n_chunks]


# ---------------------------------------------------------------------------
# Host-side constant prep
# ---------------------------------------------------------------------------
def _prep_consts(w1, b1, w2, b2, ln_n_w, ln_n_b, u1, bu1, u2, bu2,
                 ln_u_w, ln_u_b):
    f = np.float32
    A = np.stack([w1[0:128, :], w1[128:256, :]]).astype(f)
    B = np.stack([w1[256 + 0:256 + 128, :], w1[256 + 128:256 + 256, :]]).astype(f)
    W2 = np.zeros((128, 1024), f)
    for k in range(4):
        W2[:, k * 256:(k + 1) * 256] = w2[k * 128:(k + 1) * 128, :]
    U1 = np.stack([u1[0:128, :], u1[128:256, :]]).astype(f)
    U2 = np.zeros((128, 2048), f)
    for k in range(8):
        U2[:, k * 256:(k + 1) * 256] = u2[k * 128:(k + 1) * 128, :]
    Cu = w1[512:515, :]          # [3, 512]
    Cd = w1[515, :]              # [512]
    CF = np.zeros((20, 4096), f)
    for oi in range(4):
        for d in range(2):
            sgn = 1.0 if d == 0 else -1.0
            for m in range(4):
                blk = ((oi * 2 + d) * 4 + m) * 128
                CF[3 * oi:3 * oi + 3, blk:blk + 128] = \
                    sgn * Cu[:, m * 128:(m + 1) * 128]
                CF[12 + oi, blk:blk + 128] = Cd[m * 128:(m + 1) * 128]
                CF[16 + oi, blk:blk + 128] = BIG
    return dict(
        A=A, B=B, W2=W2, U1=U1, U2=U2, CF=CF,
        b1c=np.ascontiguousarray(b1.reshape(4, 128).T.astype(f)),
        bu1c=np.ascontiguousarray(bu1.reshape(8, 128).T.astype(f)),
        bu2c=np.ascontiguousarray(bu2.reshape(2, 128).T.astype(f)),
        b2r=b2.reshape(1, 256).astype(f),
        lnw=ln_n_w.reshape(1, 256).astype(f),
        lnb=ln_n_b.reshape(1, 256).astype(f),
        lnuw=ln_u_w.reshape(1, 256).astype(f),
        lnub=ln_u_b.reshape(1, 256).astype(f),
        onesH=np.full((128, 1), 1.0 / H, f),
        ones4=np.ones((4, 1), f),
        seldsq=_seldsq(), selbc=_selbc(),
        onesr=np.ones((1, 512), f),
        ident=np.eye(128, dtype=f),
        seed=np.full((128, 24), MAGIC, np.int32),
    )


def _seldsq():
    s = np.zeros((12, 4), np.float32)
    for o in range(4):
        s[3 * o:3 * o + 3, o] = 1.0
    return s


def _selbc():
    s = np.zeros((4, 12), np.float32)
    for o in range(4):
        s[o, 3 * o:3 * o + 3] = 1.0
    return s


def _run(h, xyz, valid, consts, n_chunks_per_core, core_ids, trace=False):
    from concourse.bass_utils import run_bass_kernel_spmd

    _install_patch()
    nc = _get_nc(n_chunks_per_core)
    ncore = len(core_ids)
    in_maps = []
    for i in range(ncore):
        s = slice(i * n_chunks_per_core, (i + 1) * n_chunks_per_core)
        im = dict(consts)
        im["h"] = np.ascontiguousarray(h[s])
        im["xyz"] = np.ascontiguousarray(xyz[s])
        im["valid"] = np.ascontiguousarray(valid[s])
        in_maps.append(im)
    res = run_bass_kernel_spmd(nc, in_maps, core_ids=core_ids, trace=trace)
    outs = [res.results[i]["out"] for i in range(ncore)]
    return np.concatenate(outs, axis=0), res


def kernel(h, xyz, valid, ln_n_w, ln_n_b, w1, b1, w2, b2,
           ln_u_w, ln_u_b, u1, bu1, u2, bu2):
    h = np.asarray(h, np.float32)
    xyz = np.asarray(xyz, np.float32)
    valid = np.asarray(valid).astype(np.uint8)
    consts = _prep_consts(
        np.asarray(w1), np.asarray(b1), np.asarray(w2), np.asarray(b2),
        np.asarray(ln_n_w), np.asarray(ln_n_b), np.asarray(u1),
        np.asarray(bu1), np.asarray(u2), np.asarray(bu2),
        np.asarray(ln_u_w), np.asarray(ln_u_b))
    out, _ = _run(h, xyz, valid, consts, CPC, list(range(N_CORES)))
    return out.astype(np.float32)



# revision 7
# speedup vs baseline: 1.2089x; 1.2089x over previous
"""LocalGraphMessageBlock TRN2 kernel (v2, bf16 matmuls).

Math (per chunk of C=512 tokens, H=256 features, offsets 1,2,4,8):
  h_in = LN(h);  per offset o and direction:
    z = P[dst] + Q[src] + C5 @ e  (P/Q added on the PE via identity
        matmuls into the same PSUM accumulation as the e-matmul;
        -BIG*(1-mask) folded into the e-matmul so gelu(z)=0 on invalid
        edges; b1 folded into P)
    agg += gelu(z) @ w2           (PSUM accumulation across all 8 dirs)
  agg += b2 (x) deg               (outer product, deg = #valid edges/dst)
  h2 = h + agg;  out = (h2 + MLP(LN(h2))) * valid

All matmuls run in bf16 (PSUM accumulates fp32). LayerNorm weights/
biases are folded into the adjacent MLP weights host-side, so on-chip
LN is just (x-mu)*rsqrt(var+eps) with rsqrt = ACT Sqrt + DVE
reciprocal_approx_fast. h loads feature-major straight from DRAM (no
PE in-transposes); the output goes back token-major via bf16 PE
transposes with the valid-mask applied as a per-partition ACT scale.

Data-parallel over the chunk dim N: 256 chunks / 8 cores = 32 chunks
each, same NEFF, per-core input slices.
"""
import json

import numpy as np

N_TOT, C, H = 256, 512, 256
OFFSETS = (1, 2, 4, 8)
N_CORES = 8
CPC = N_TOT // N_CORES  # chunks per core
BIG = 30000.0
EPS = 1e-5

# ---------------------------------------------------------------------------
# Walrus workaround: this container's walrus accepts at most ONE sync-wait
# command per instruction; Tile emits more. Split excess onto preceding
# NoOps on the same engine (engine queues are in-order, so this is
# equivalent gating).
# ---------------------------------------------------------------------------
_patched = False


def _split_sync_waits(bir_json: bytes, maxw: int = 1) -> bytes:
    m = json.loads(bir_json)
    cnt = 0
    changed = False
    for f in m.get("functions", []):
        for blk in f.get("blocks", []):
            newins = []
            for ins in blk.get("instructions", []):
                si = ins.get("sync_info")
                if si:
                    waits = si.get("on_wait") or []
                    if len(waits) > maxw:
                        changed = True
                        si["on_wait"] = waits[-maxw:]
                        extra = waits[:-maxw]
                        for i in range(0, len(extra), maxw):
                            cnt += 1
                            newins.append({
                                "debug": ins.get("debug", 0),
                                "engine": ins["engine"],
                                "ins": [], "outs": [],
                                "name": f"{ins['name']}-ws{cnt}",
                                "opcode": "NoOp",
                                "sync_info": {"on_update": [],
                                              "on_wait": extra[i:i + maxw]},
                            })
                newins.append(ins)
            blk["instructions"] = newins
    return json.dumps(m).encode() if changed else bir_json


def _install_patch():
    global _patched
    if _patched:
        return
    import concourse.bass_utils as bu
    import concourse.bass2jax as b2j

    orig = bu.compile_bir_kernel

    def patched(bir_json, tmpdir, neff_name="file.neff"):
        return orig(_split_sync_waits(bir_json), tmpdir, neff_name)

    bu.compile_bir_kernel = patched
    b2j.compile_bir_kernel = patched
    _patched = True


# ---------------------------------------------------------------------------
# Bass kernel builder
# ---------------------------------------------------------------------------
_nc_cache = {}


def _build(n_chunks):
    import concourse.bass as bass
    import concourse.tile as tile
    from concourse import mybir

    f32 = mybir.dt.float32
    bf16 = mybir.dt.bfloat16
    u8 = mybir.dt.uint8
    AF = mybir.ActivationFunctionType
    ALU = mybir.AluOpType

    nc = bass.Bass("TRN2")

    # ---- dram I/O ----
    h_d = nc.dram_tensor("h", [n_chunks, C, H], f32, kind="ExternalInput")
    xyz_d = nc.dram_tensor("xyz", [n_chunks, C, 3], f32, kind="ExternalInput")
    val_d = nc.dram_tensor("valid", [n_chunks, C], u8, kind="ExternalInput")
    out_d = nc.dram_tensor("out", [n_chunks, C, H], f32, kind="ExternalOutput")

    def din(name, shape, dt=bf16):
        return nc.dram_tensor(name, shape, dt, kind="ExternalInput")

    A_d = din("A", [2, 128, 512])       # w1[:256] k-halves  (lhsT blocks)
    B_d = din("B", [2, 128, 512])       # w1[256:512]
    W2_d = din("W2", [128, 1024])       # w2 k-blocks: [:, k*256:(k+1)*256]
    U1_d = din("U1", [2, 128, 1024])
    U2_d = din("U2", [128, 2048])       # u2 k-blocks: [:, k*256:(k+1)*256]
    CF_d = din("CF", [20, 4096])        # e-matmul lhsT blocks (oi,dir,m)
    b1c_d = din("b1c", [128, 4], f32)
    bu1c_d = din("bu1c", [128, 8], f32)
    bu2c_d = din("bu2c", [128, 2], f32)
    b2r_d = din("b2r", [1, 256])
    onesH_d = din("onesH", [128, 1])    # 1/H
    ones1_d = din("ones1", [1, 128])    # ones (broadcast lhsT)
    ones4_d = din("ones4", [4, 1])
    seldsq_d = din("seldsq", [12, 4])
    selbc_d = din("selbc", [4, 12])
    ident_d = din("ident", [128, 128])
    epsd_d = din("epsd", [4, 1], f32)
    epsv_d = din("epsv", [1, 1], f32)

    from contextlib import ExitStack
    with tile.TileContext(nc) as tc, ExitStack() as ctx:
        cp = ctx.enter_context(tc.tile_pool(name="consts", bufs=1))
        ld = ctx.enter_context(tc.tile_pool(name="loads", bufs=2))
        wk = ctx.enter_context(tc.tile_pool(name="work", bufs=2))
        sm = ctx.enter_context(tc.tile_pool(name="small", bufs=2))
        pz = ctx.enter_context(tc.tile_pool(name="pz", bufs=2, space="PSUM"))
        pmm = ctx.enter_context(tc.tile_pool(name="pmm", bufs=2, space="PSUM"))
        pagg = ctx.enter_context(tc.tile_pool(name="pagg", bufs=1, space="PSUM"))
        pms = ctx.enter_context(tc.tile_pool(name="pms", bufs=2, space="PSUM"))

        # ---- load constants ----
        def cload(dram, shape, dt=bf16, name=None):
            t = cp.tile(shape, dt, name=name, tag=name)
            nc.sync.dma_start(t, dram[tuple(slice(None) for _ in shape)])
            return t

        A0 = cp.tile([128, 512], bf16); nc.sync.dma_start(A0, A_d[0])
        A1 = cp.tile([128, 512], bf16); nc.sync.dma_start(A1, A_d[1])
        B0 = cp.tile([128, 512], bf16); nc.sync.dma_start(B0, B_d[0])
        B1 = cp.tile([128, 512], bf16); nc.sync.dma_start(B1, B_d[1])
        W2 = cload(W2_d, [128, 1024], name="W2")
        U1a = cp.tile([128, 1024], bf16); nc.sync.dma_start(U1a, U1_d[0])
        U1b = cp.tile([128, 1024], bf16); nc.sync.dma_start(U1b, U1_d[1])
        U2 = cload(U2_d, [128, 2048], name="U2")
        CF = cload(CF_d, [20, 4096], name="CF")
        b1c = cload(b1c_d, [128, 4], f32, name="b1c")
        bu1c = cload(bu1c_d, [128, 8], f32, name="bu1c")
        bu2c = cload(bu2c_d, [128, 2], f32, name="bu2c")
        b2r = cload(b2r_d, [1, 256], name="b2r")
        onesH = cload(onesH_d, [128, 1], name="onesH")
        ones1 = cload(ones1_d, [1, 128], name="ones1")
        ones4 = cload(ones4_d, [4, 1], name="ones4")
        seldsq = cload(seldsq_d, [12, 4], name="seldsq")
        selbc = cload(selbc_d, [4, 12], name="selbc")
        ident = cload(ident_d, [128, 128], name="ident")
        epsd = cload(epsd_d, [4, 1], f32, name="epsd")
        epsv = cload(epsv_d, [1, 1], f32, name="epsv")

        def ln_rows(xb, tag):
            """Feature-major LN stats of xb (2x [128,512] bf16).

            Returns (rstd_b, sh_b): bf16 [1,512] rows with
            rstd = 1/sqrt(var+eps), sh = -mu*rstd."""
            mu_ps = pms.tile([1, 512], f32, name="mu_ps", tag="miscps")
            m2_ps = pms.tile([1, 512], f32, name="m2_ps", tag="miscps")
            for fh in range(2):
                x2 = wk.tile([128, 512], bf16, name=f"x2{tag}", tag=f"x2{tag}")
                nc.gpsimd.tensor_mul(x2, xb[fh], xb[fh])
                nc.tensor.matmul(mu_ps, onesH, xb[fh], start=(fh == 0),
                                 stop=(fh == 1))
                nc.tensor.matmul(m2_ps, onesH, x2, start=(fh == 0),
                                 stop=(fh == 1))
            mumu = sm.tile([1, 512], f32, name=f"mumu{tag}", tag=f"mumu{tag}")
            nc.scalar.activation(mumu, mu_ps, AF.Square)
            vare = sm.tile([1, 512], f32, name=f"var{tag}", tag=f"var{tag}")
            nc.vector.tensor_sub(vare, m2_ps, mumu)
            sd = sm.tile([1, 512], f32, name=f"sd{tag}", tag=f"sd{tag}")
            nc.scalar.activation(sd, vare, AF.Sqrt, bias=epsv[0:1])
            rstd = sm.tile([1, 512], f32, name=f"rstd{tag}", tag=f"rstd{tag}")
            nc.vector.reciprocal(out=rstd, in_=sd)
            rstd_b = sm.tile([1, 512], bf16, name=f"rstdb{tag}",
                             tag=f"rstdb{tag}")
            nc.vector.tensor_copy(rstd_b, rstd)
            sh_b = sm.tile([1, 512], bf16, name=f"shb{tag}", tag=f"shb{tag}")
            nc.vector.scalar_tensor_tensor(out=sh_b, in0=mu_ps, scalar=-1.0,
                                           in1=rstd, op0=ALU.mult,
                                           op1=ALU.mult)
            return rstd_b, sh_b

        def ln_apply(xb, rstd_b, sh_b, tag):
            """out[fh] = xb[fh]*bcast(rstd) + bcast(sh), bf16."""
            arep = pms.tile([128, 512], f32, name="arep", tag="miscps")
            nc.tensor.matmul(arep, ones1, rstd_b, start=True, stop=True)
            brep = pms.tile([128, 512], f32, name="brep", tag="miscps")
            nc.tensor.matmul(brep, ones1, sh_b, start=True, stop=True)
            outs = []
            for fh in range(2):
                o = wk.tile([128, 512], bf16, name=f"ln{tag}{fh}",
                            tag=f"ln{tag}{fh}", bufs=1)
                nc.vector.tensor_mul(o, xb[fh], arep)
                nc.vector.tensor_add(o, o, brep)
                outs.append(o)
            return outs

        for ci in range(n_chunks):
            # ---------------- loads ----------------
            hb = h_d[ci]
            hfm = []
            for fh in range(2):
                t = ld.tile([128, 512], f32, name=f"hfm{fh}", tag=f"hfm{fh}")
                nc.sync.dma_start(
                    t, bass.AP(tensor=hb.tensor, offset=hb.offset + fh * 128,
                               ap=[[1, 128], [256, 512]]))
                hfm.append(t)
            hfmb = []
            for fh in range(2):
                t = wk.tile([128, 512], bf16, name=f"hfmb{fh}", tag=f"hfmb{fh}")
                nc.vector.tensor_copy(t, hfm[fh])
                hfmb.append(t)
            xyzp = sm.tile([3, 520], f32, name="xyzp", tag="xyzp")
            nc.vector.memset(xyzp, 0.0)
            nc.sync.dma_start(xyzp[:, 0:512],
                              xyz_d[ci].rearrange("t k -> k t"))
            vbase = val_d[ci]
            vr_u8 = sm.tile([4, 512], u8, name="vru", tag="vru")
            nc.sync.dma_start(
                vr_u8, bass.AP(tensor=vbase.tensor, offset=vbase.offset,
                               ap=[[0, 4], [1, 512]]))
            vrf = sm.tile([4, 512], bf16, name="vrf", tag="vrf")
            nc.vector.tensor_copy(vrf, vr_u8)
            vrs_u8 = sm.tile([4, 512], u8, name="vrsu", tag="vrsu")
            nc.vector.memset(vrs_u8, 0)
            for oi, off in enumerate(OFFSETS):
                nc.sync.dma_start(vrs_u8[oi:oi + 1, 0:C - off],
                                  val_d[ci, off:C][None, :])
            vrsf = sm.tile([4, 512], bf16, name="vrsf", tag="vrsf")
            nc.vector.tensor_copy(vrsf, vrs_u8)
            vcol_u8 = sm.tile([128, 4], u8, name="vcu", tag="vcu")
            nc.sync.dma_start(
                vcol_u8, bass.AP(tensor=vbase.tensor, offset=vbase.offset,
                                 ap=[[1, 128], [128, 4]]))
            vcolf = sm.tile([128, 4], f32, name="vcf", tag="vcf")
            nc.vector.tensor_copy(vcolf, vcol_u8)

            # ---------------- LN1 ----------------
            rstd1, sh1 = ln_rows(hfmb, "a")
            hin = ln_apply(hfmb, rstd1, sh1, "a")

            # ---------------- edge features ----------------
            delta = sm.tile([12, 512], bf16, name="delta", tag="delta")
            for oi, off in enumerate(OFFSETS):
                dlo = sm.tile([3, 512], bf16, name=f"dlo{oi}", tag=f"dlo{oi}")
                nc.vector.tensor_sub(dlo, xyzp[:, off:off + 512],
                                     xyzp[:, 0:512])
                nc.sync.dma_start(delta[3 * oi:3 * oi + 3], dlo)
            dsq = sm.tile([12, 512], bf16, name="dsq", tag="dsq")
            nc.gpsimd.tensor_mul(dsq, delta, delta)
            d2_ps = pms.tile([4, 512], f32, name="d2_ps", tag="miscps")
            nc.tensor.matmul(d2_ps, seldsq, dsq, start=True, stop=True)
            dist4 = sm.tile([4, 512], f32, name="dist4", tag="dist4")
            nc.scalar.activation(dist4, d2_ps, AF.Sqrt, bias=epsd[:, 0:1])
            invd4 = sm.tile([4, 512], f32, name="invd4", tag="invd4")
            nc.vector.reciprocal(out=invd4, in_=dist4)
            invd4b = sm.tile([4, 512], bf16, name="invd4b", tag="invd4b")
            nc.vector.tensor_copy(invd4b, invd4)
            dist4b = sm.tile([4, 512], bf16, name="dist4b", tag="dist4b")
            nc.vector.tensor_copy(dist4b, dist4)

            e20 = sm.tile([20, 512], bf16, name="e20", tag="e20")
            nc.sync.dma_start(e20[12:16], dist4b)
            m_all = sm.tile([4, 512], bf16, name="mall", tag="mall")
            nc.gpsimd.tensor_mul(m_all, vrf, vrsf)
            mm1 = sm.tile([4, 512], bf16, name="mm1", tag="mm1")
            nc.vector.tensor_scalar(out=mm1, in0=m_all, scalar1=1.0,
                                    scalar2=None, op0=ALU.subtract)
            nc.sync.dma_start(e20[16:20], mm1)
            invrep = pms.tile([12, 512], f32, name="invrep", tag="miscps")
            nc.tensor.matmul(invrep, selbc, invd4b, start=True, stop=True)
            nc.vector.tensor_mul(e20[0:12], delta, invrep)

            # ---------------- degree / agg init ----------------
            mrev = sm.tile([4, 512], bf16, name="mrev", tag="mrev")
            nc.vector.memset(mrev, 0.0)
            for oi, off in enumerate(OFFSETS):
                nc.sync.dma_start(mrev[oi:oi + 1, off:C],
                                  m_all[oi:oi + 1, 0:C - off])
            deg_ps = pms.tile([1, 512], f32, name="deg_ps", tag="miscps")
            nc.tensor.matmul(deg_ps, ones4, m_all, start=True, stop=False)
            nc.tensor.matmul(deg_ps, ones4, mrev, start=False, stop=True)
            deg_row = sm.tile([1, 512], bf16, name="degr", tag="degr")
            nc.vector.tensor_copy(deg_row, deg_ps)

            agg = [pagg.tile([128, 512], f32, name=f"agg{fh}", tag=f"agg{fh}")
                   for fh in range(2)]
            for fh in range(2):
                nc.tensor.matmul(agg[fh], b2r[:, fh * 128:(fh + 1) * 128],
                                 deg_row, start=True, stop=False,
                                 skip_group_check=True)

            # ---------------- P, Q ----------------
            P = wk.tile([128, 2048], bf16, name="P", tag="P")
            Q = wk.tile([128, 2048], bf16, name="Q", tag="Q")
            for m in range(4):
                pq = pmm.tile([128, 512], f32, name="mmps", tag="mmps")
                nc.tensor.matmul(pq, A0[:, m * 128:(m + 1) * 128], hin[0],
                                 start=True, stop=False)
                nc.tensor.matmul(pq, A1[:, m * 128:(m + 1) * 128], hin[1],
                                 start=False, stop=True)
                nc.vector.tensor_scalar(out=P[:, m * 512:(m + 1) * 512],
                                        in0=pq, scalar1=b1c[:, m:m + 1],
                                        scalar2=None, op0=ALU.add)
                pq2 = pmm.tile([128, 512], f32, name="mmps", tag="mmps")
                nc.tensor.matmul(pq2, B0[:, m * 128:(m + 1) * 128], hin[0],
                                 start=True, stop=False)
                nc.tensor.matmul(pq2, B1[:, m * 128:(m + 1) * 128], hin[1],
                                 start=False, stop=True)
                nc.vector.tensor_copy(Q[:, m * 512:(m + 1) * 512], pq2)

            # ---------------- messages ----------------
            for oi, off in enumerate(OFFSETS):
                N = C - off
                for d in range(2):  # 0=fwd (dst=i), 1=rev (dst=j)
                    po = 0 if d == 0 else off
                    qo = off if d == 0 else 0
                    tzg = wk.tile([128, 2048], bf16, name="tzg", tag="tzg")
                    for m in range(4):
                        blk = ((oi * 2 + d) * 4 + m) * 128
                        ze = pz.tile([128, 512], f32, name="zps", tag="zps")
                        nc.tensor.matmul(ze[:, 0:N], CF[:, blk:blk + 128],
                                         e20[:, 0:N], start=True, stop=False)
                        nc.tensor.matmul(
                            ze[:, 0:N], ident,
                            P[:, m * 512 + po:m * 512 + po + N],
                            start=False, stop=False)
                        nc.tensor.matmul(
                            ze[:, 0:N], ident,
                            Q[:, m * 512 + qo:m * 512 + qo + N],
                            start=False, stop=True)
                        nc.scalar.activation(tzg[:, m * 512:m * 512 + N],
                                             ze[:, 0:N], AF.Gelu)
                    for fh in range(2):
                        dst = (agg[fh][:, 0:N] if d == 0
                               else agg[fh][:, off:off + N])
                        for k in range(4):
                            last = (oi == 3 and d == 1 and k == 3)
                            nc.tensor.matmul(
                                dst, W2[:, k * 256 + fh * 128:
                                        k * 256 + (fh + 1) * 128],
                                tzg[:, k * 512:k * 512 + N],
                                start=False, stop=last,
                                skip_group_check=True)

            # ---------------- update MLP ----------------
            h2b = [wk.tile([128, 512], bf16, name=f"h2{fh}", tag=f"h2{fh}",
                           bufs=1) for fh in range(2)]
            for fh in range(2):
                nc.vector.tensor_add(h2b[fh], hfm[fh], agg[fh])
            rstd2, sh2 = ln_rows(h2b, "u")
            hun = ln_apply(h2b, rstd2, sh2, "u")
            gu = wk.tile([128, 4096], bf16, name="gu", tag="gu", bufs=1)
            for um in range(8):
                up = pmm.tile([128, 512], f32, name="mmps", tag="mmps")
                nc.tensor.matmul(up, U1a[:, um * 128:(um + 1) * 128], hun[0],
                                 start=True, stop=False)
                nc.tensor.matmul(up, U1b[:, um * 128:(um + 1) * 128], hun[1],
                                 start=False, stop=True)
                nc.scalar.activation(gu[:, um * 512:(um + 1) * 512],
                                     up, AF.Gelu, bias=bu1c[:, um:um + 1])
            of = [wk.tile([128, 512], bf16, name=f"of{fh}", tag=f"of{fh}",
                          bufs=1) for fh in range(2)]
            for fh in range(2):
                u2p = pmm.tile([128, 512], f32, name="mmps", tag="mmps")
                for k in range(8):
                    nc.tensor.matmul(
                        u2p, U2[:, k * 256 + fh * 128:k * 256 + (fh + 1) * 128],
                        gu[:, k * 512:(k + 1) * 512],
                        start=(k == 0), stop=(k == 7))
                nc.vector.scalar_tensor_tensor(
                    out=of[fh], in0=u2p, scalar=bu2c[:, fh:fh + 1],
                    in1=h2b[fh], op0=ALU.add, op1=ALU.add)

            # ---------------- transpose out + mask + store ----------------
            osb = ld.tile([128, 1024], f32, name="osb", tag="osb")
            for i in range(4):
                for fh in range(2):
                    tp = pms.tile([128, 128], bf16, name="tp", tag="miscps")
                    nc.tensor.transpose(tp, of[fh][:, i * 128:(i + 1) * 128],
                                        ident)
                    nc.scalar.activation(
                        osb[:, i * 256 + fh * 128:i * 256 + (fh + 1) * 128],
                        tp, AF.Copy, bias=0.0, scale=vcolf[:, i:i + 1])
            ob = out_d[ci]
            nc.sync.dma_start(
                bass.AP(tensor=ob.tensor, offset=ob.offset,
                        ap=[[256, 128], [128 * 256, 4], [1, 256]]),
                osb.rearrange("p (i f) -> p i f", i=4))

    return nc


def _get_nc(n_chunks):
    if n_chunks not in _nc_cache:
        _nc_cache[n_chunks] = _build(n_chunks)
    return _nc_cache[n_chunks]


# ---------------------------------------------------------------------------
# Host-side constant prep
# ---------------------------------------------------------------------------
def _prep_consts(w1, b1, w2, b2, ln_n_w, ln_n_b, u1, bu1, u2, bu2,
                 ln_u_w, ln_u_b):
    import ml_dtypes
    f = np.float32
    bf = ml_dtypes.bfloat16
    w1 = np.asarray(w1, f)
    u1 = np.asarray(u1, f)
    ln_n_w = np.asarray(ln_n_w, f)
    ln_n_b = np.asarray(ln_n_b, f)
    ln_u_w = np.asarray(ln_u_w, f)
    ln_u_b = np.asarray(ln_u_b, f)
    # Fold LN1 weight/bias into w1's h-halves / b1, LN2's into u1 / bu1.
    w1f = w1.copy()
    w1f[0:256] *= ln_n_w[:, None]
    w1f[256:512] *= ln_n_w[:, None]
    b1f = (np.asarray(b1, f) + ln_n_b @ w1[0:256] + ln_n_b @ w1[256:512])
    u1f = u1 * ln_u_w[:, None]
    bu1f = np.asarray(bu1, f) + ln_u_b @ u1

    A = np.stack([w1f[0:128, :], w1f[128:256, :]]).astype(bf)
    B = np.stack([w1f[256:384, :], w1f[384:512, :]]).astype(bf)
    W2 = np.zeros((128, 1024), f)
    for k in range(4):
        W2[:, k * 256:(k + 1) * 256] = w2[k * 128:(k + 1) * 128, :]
    U1 = np.stack([u1f[0:128, :], u1f[128:256, :]]).astype(bf)
    U2 = np.zeros((128, 2048), f)
    for k in range(8):
        U2[:, k * 256:(k + 1) * 256] = u2[k * 128:(k + 1) * 128, :]
    Cu = w1[512:515, :]          # [3, 512]
    Cd = w1[515, :]              # [512]
    CF = np.zeros((20, 4096), f)
    for oi in range(4):
        for d in range(2):
            sgn = 1.0 if d == 0 else -1.0
            for m in range(4):
                blk = ((oi * 2 + d) * 4 + m) * 128
                CF[3 * oi:3 * oi + 3, blk:blk + 128] = \
                    sgn * Cu[:, m * 128:(m + 1) * 128]
                CF[12 + oi, blk:blk + 128] = Cd[m * 128:(m + 1) * 128]
                CF[16 + oi, blk:blk + 128] = BIG
    return dict(
        A=A, B=B, W2=W2.astype(bf), U1=U1, U2=U2.astype(bf), CF=CF.astype(bf),
        b1c=np.ascontiguousarray(b1f.reshape(4, 128).T.astype(f)),
        bu1c=np.ascontiguousarray(bu1f.reshape(8, 128).T.astype(f)),
        bu2c=np.ascontiguousarray(np.asarray(bu2, f).reshape(2, 128).T),
        b2r=np.asarray(b2, f).reshape(1, 256).astype(bf),
        onesH=np.full((128, 1), 1.0 / H, bf),
        ones1=np.ones((1, 128), bf),
        ones4=np.ones((4, 1), bf),
        seldsq=_seldsq().astype(bf), selbc=_selbc().astype(bf),
        ident=np.eye(128, dtype=f).astype(bf),
        epsd=np.full((4, 1), 1e-12, f),
        epsv=np.full((1, 1), EPS, f),
    )


def _seldsq():
    s = np.zeros((12, 4), np.float32)
    for o in range(4):
        s[3 * o:3 * o + 3, o] = 1.0
    return s


def _selbc():
    s = np.zeros((4, 12), np.float32)
    for o in range(4):
        s[o, 3 * o:3 * o + 3] = 1.0
    return s


def _run(h, xyz, valid, consts, n_chunks_per_core, core_ids, trace=False):
    from concourse.bass_utils import run_bass_kernel_spmd

    _install_patch()
    nc = _get_nc(n_chunks_per_core)
    ncore = len(core_ids)
    in_maps = []
    for i in range(ncore):
        s = slice(i * n_chunks_per_core, (i + 1) * n_chunks_per_core)
        im = dict(consts)
        im["h"] = np.ascontiguousarray(h[s])
        im["xyz"] = np.ascontiguousarray(xyz[s])
        im["valid"] = np.ascontiguousarray(valid[s])
        in_maps.append(im)
    res = run_bass_kernel_spmd(nc, in_maps, core_ids=core_ids, trace=trace)
    outs = [res.results[i]["out"] for i in range(ncore)]
    return np.concatenate(outs, axis=0), res


def kernel(h, xyz, valid, ln_n_w, ln_n_b, w1, b1, w2, b2,
           ln_u_w, ln_u_b, u1, bu1, u2, bu2):
    h = np.asarray(h, np.float32)
    xyz = np.asarray(xyz, np.float32)
    valid = np.asarray(valid).astype(np.uint8)
    consts = _prep_consts(
        np.asarray(w1), np.asarray(b1), np.asarray(w2), np.asarray(b2),
        np.asarray(ln_n_w), np.asarray(ln_n_b), np.asarray(u1),
        np.asarray(bu1), np.asarray(u2), np.asarray(bu2),
        np.asarray(ln_u_w), np.asarray(ln_u_b))
    out, _ = _run(h, xyz, valid, consts, CPC, list(range(N_CORES)))
    return out.astype(np.float32)


# revision 16
# speedup vs baseline: 3.8056x; 3.1481x over previous
"""LocalGraphMessageBlock TRN2 kernel (bf16/fp8 matmuls, software-pipelined).

Math (per chunk of C=512 tokens, H=256 features, offsets 1,2,4,8):
  h_in = LN(h);  per offset o and direction:
    z = (P[dst] + Q[src]) + C5 @ e   (P+Q precomputed on DVE/GpSimd; added
        into the e-matmul's PSUM group via one bf16 identity matmul;
        -BIG*(1-mask) folded into the e-matmul so gelu(z)=0 on invalid
        edges; b1 and the LN weights folded into P / the w1 weights)
    agg += gelu(z) @ w2              (PSUM accumulation across all 8 dirs)
  agg += b2 (x) deg                  (outer product, deg = #valid edges/dst)
  h2 = h + agg;  out = (h2 + MLP(LN(h2))) * valid

Precision: matmuls in bf16 except the update MLP (u1/u2) which runs
fp8e4 DoubleRow (2 contraction tiles per instruction); PSUM accumulates
fp32 everywhere. rsqrt = bit-hack seed + 1 Newton step on DVE, with the
rows transpose-packed to [128, 4r] via the PE so the DVE ops are short.

Schedule: the per-chunk front phase (loads, LN1, edge features, P/Q) is
emitted as generator stages interleaved between the previous chunk's
message direction-blocks, and the token-major output transpose/store of
chunk i-1 fills the LN2 rsqrt window of chunk i. This keeps the PE ~90%
busy despite the serial DVE/ACT chains.

Data-parallel over the chunk dim N: 256 chunks / 8 cores = 32 chunks
each, same NEFF, per-core input slices.
"""
import json

import numpy as np

N_TOT, C, H = 256, 512, 256
OFFSETS = (1, 2, 4, 8)
N_CORES = 8
CPC = N_TOT // N_CORES  # chunks per core
BIG = 30000.0
EPS = 1e-5
MAGIC = 0x5F3759DF

# ---------------------------------------------------------------------------
# Walrus workaround: this container's walrus accepts at most ONE sync-wait
# command per instruction; Tile emits more. Split excess onto preceding
# NoOps on the same engine (engine queues are in-order, so this is
# equivalent gating).
# ---------------------------------------------------------------------------
_patched = False


def _split_sync_waits(bir_json: bytes, maxw: int = 1) -> bytes:
    m = json.loads(bir_json)
    cnt = 0
    changed = False
    for f in m.get("functions", []):
        for blk in f.get("blocks", []):
            newins = []
            for ins in blk.get("instructions", []):
                si = ins.get("sync_info")
                if si:
                    waits = si.get("on_wait") or []
                    if len(waits) > maxw:
                        changed = True
                        si["on_wait"] = waits[-maxw:]
                        extra = waits[:-maxw]
                        for i in range(0, len(extra), maxw):
                            cnt += 1
                            newins.append({
                                "debug": ins.get("debug", 0),
                                "engine": ins["engine"],
                                "ins": [], "outs": [],
                                "name": f"{ins['name']}-ws{cnt}",
                                "opcode": "NoOp",
                                "sync_info": {"on_update": [],
                                              "on_wait": extra[i:i + maxw]},
                            })
                newins.append(ins)
            blk["instructions"] = newins
    return json.dumps(m).encode() if changed else bir_json


def _install_patch():
    global _patched
    if _patched:
        return
    import concourse.bass_utils as bu
    import concourse.bass2jax as b2j

    orig = bu.compile_bir_kernel

    def patched(bir_json, tmpdir, neff_name="file.neff"):
        return orig(_split_sync_waits(bir_json), tmpdir, neff_name)

    bu.compile_bir_kernel = patched
    b2j.compile_bir_kernel = patched
    _patched = True


# ---------------------------------------------------------------------------
# Bass kernel builder
# ---------------------------------------------------------------------------
_nc_cache = {}


def _build(n_chunks):
    import concourse.bass as bass
    import concourse.tile as tile
    from concourse import mybir

    f32 = mybir.dt.float32
    bf16 = mybir.dt.bfloat16
    u8 = mybir.dt.uint8
    AF = mybir.ActivationFunctionType
    ALU = mybir.AluOpType

    nc = bass.Bass("TRN2")

    # ---- dram I/O ----
    h_d = nc.dram_tensor("h", [n_chunks, C, H], f32, kind="ExternalInput")
    xyz_d = nc.dram_tensor("xyz", [n_chunks, C, 3], f32, kind="ExternalInput")
    val_d = nc.dram_tensor("valid", [n_chunks, C], u8, kind="ExternalInput")
    out_d = nc.dram_tensor("out", [n_chunks, C, H], f32, kind="ExternalOutput")

    def din(name, shape, dt=bf16):
        return nc.dram_tensor(name, shape, dt, kind="ExternalInput")

    A_d = din("A", [128, 1024])         # [p, (h 2)(out 512)]
    B_d = din("B", [128, 1024])
    W2_d = din("W2", [128, 1024])       # w2 k-blocks: [:, k*256:(k+1)*256]
    U1_d = din("U1", [128, 2048], mybir.dt.float8e4)   # [p, (h 2)(out 1024)]
    U2_d = din("U2", [128, 2048], mybir.dt.float8e4)   # u2 k-blocks (fp8)
    CF_d = din("CF", [20, 4096])        # e-matmul lhsT blocks (oi,dir,m)
    b1c_d = din("b1c", [128, 4], f32)
    bu1c_d = din("bu1c", [128, 8], f32)
    bu2c_d = din("bu2c", [128, 2], f32)
    b2r_d = din("b2r", [1, 256])
    onesH_d = din("onesH", [128, 1])    # 1/H
    ones1_d = din("ones1", [1, 128])    # ones (broadcast lhsT)
    ones4_d = din("ones4", [4, 1])
    seldsq_d = din("seldsq", [12, 4])
    selbc_d = din("selbc", [4, 12])
    ident_d = din("ident", [128, 128])
    seed_d = din("seed", [128, 24], nc_i32 := mybir.dt.int32)
    ident32_d = din("ident32", [128, 128], f32)

    from contextlib import ExitStack
    with tile.TileContext(nc) as tc, ExitStack() as ctx:
        cp = ctx.enter_context(tc.tile_pool(name="consts", bufs=1))
        ld = ctx.enter_context(tc.tile_pool(name="loads", bufs=2))
        wk = ctx.enter_context(tc.tile_pool(name="work", bufs=2))
        sm = ctx.enter_context(tc.tile_pool(name="small", bufs=2))
        pz = ctx.enter_context(tc.tile_pool(name="pz", bufs=2, space="PSUM"))
        pmm = ctx.enter_context(tc.tile_pool(name="pmm", bufs=2, space="PSUM"))
        pagg = ctx.enter_context(tc.tile_pool(name="pagg", bufs=1, space="PSUM"))
        pms = ctx.enter_context(tc.tile_pool(name="pms", bufs=2, space="PSUM"))

        # ---- load constants ----
        def cload(dram, shape, dt=bf16, name=None):
            t = cp.tile(shape, dt, name=name, tag=name)
            nc.sync.dma_start(t, dram[tuple(slice(None) for _ in shape)])
            return t

        At = cload(A_d, [128, 1024], name="At")
        Bt = cload(B_d, [128, 1024], name="Bt")
        W2 = cload(W2_d, [128, 1024], name="W2")
        U1t = cload(U1_d, [128, 2048], f8, name="U1t")
        U1r = U1t.rearrange("p (h c) -> p h c", h=2)
        U2 = cload(U2_d, [128, 2048], f8, name="U2")
        U2r = U2.rearrange("p (k g) -> p k g", k=8)
        CF = cload(CF_d, [20, 4096], name="CF")
        b1c = cload(b1c_d, [128, 4], f32, name="b1c")
        bu1c = cload(bu1c_d, [128, 8], f32, name="bu1c")
        bu2c = cload(bu2c_d, [128, 2], f32, name="bu2c")
        b2r = cload(b2r_d, [1, 256], name="b2r")
        onesH = cload(onesH_d, [128, 1], name="onesH")
        ones1 = cload(ones1_d, [1, 128], name="ones1")
        ones4 = cload(ones4_d, [4, 1], name="ones4")
        seldsq = cload(seldsq_d, [12, 4], name="seldsq")
        selbc = cload(selbc_d, [4, 12], name="selbc")
        ident = cload(ident_d, [128, 128], name="ident")
        seed = cload(seed_d, [128, 24], nc_i32, name="seed")
        ident32 = cload(ident32_d, [128, 128], f32, name="ident32")

        def rsqrt_rows(rows, r, tag):
            """rows: [r, 512] sbuf fp32 (positive) -> [r, 512] fp32 of 1/sqrt.

            Bit-hack seed + 3 Newton steps on DVE, transpose-packed to
            [128, 4r] via the PE so the DVE ops are short."""
            i32 = nc_i32
            w = 4 * r
            rt = pms.tile([128, w], f32, name="rt", tag="miscps")
            for g in range(4):
                nc.tensor.transpose(rt[:, g * r:(g + 1) * r],
                                    rows[:, g * 128:(g + 1) * 128],
                                    ident32[0:r, 0:r])
            x = sm.tile([128, w], f32, name=f"nrx{tag}", tag=f"nrx{tag}")
            nc.vector.tensor_copy(x, rt)
            yi = sm.tile([128, w], i32, name=f"nry{tag}", tag=f"nry{tag}")
            nc.vector.tensor_scalar(out=yi, in0=x.bitcast(i32), scalar1=1,
                                    scalar2=None, op0=ALU.logical_shift_right)
            nc.vector.tensor_sub(yi, seed[:, 0:w], yi)
            y = yi.bitcast(f32)
            t = sm.tile([128, w], f32, name=f"nrt{tag}", tag=f"nrt{tag}")
            for _ in range(3):
                nc.vector.tensor_mul(t, y, y)
                nc.vector.tensor_mul(t, t, x)
                nc.vector.tensor_scalar(out=t, in0=t, scalar1=-0.5,
                                        scalar2=1.5, op0=ALU.mult, op1=ALU.add)
                nc.vector.tensor_mul(y, y, t)
            rp = pms.tile([r, 512], f32, name="rp", tag="miscps")
            for g in range(4):
                nc.tensor.transpose(rp[:, g * 128:(g + 1) * 128],
                                    y[:, g * r:(g + 1) * r], ident32)
            out = sm.tile([r, 512], f32, name=f"nro{tag}", tag=f"nro{tag}")
            nc.vector.tensor_copy(out, rp)
            return out

        def ln_rows(xb, tag):
            """Feature-major LN stats of xb (2x [128,512] bf16).

            Returns (rstd_b, sh_b): bf16 [1,512] rows with
            rstd = 1/sqrt(var+eps), sh = -mu*rstd."""
            mu_ps = pms.tile([1, 512], f32, name="mu_ps", tag="miscps")
            m2_ps = pms.tile([1, 512], f32, name="m2_ps", tag="miscps")
            for fh in range(2):
                x2 = wk.tile([128, 512], bf16, name=f"x2{tag}", tag=f"x2{tag}")
                nc.gpsimd.tensor_mul(x2, xb[fh], xb[fh])
                nc.tensor.matmul(mu_ps, onesH, xb[fh], start=(fh == 0),
                                 stop=(fh == 1))
                nc.tensor.matmul(m2_ps, onesH, x2, start=(fh == 0),
                                 stop=(fh == 1))
            mumu = sm.tile([1, 512], f32, name=f"mumu{tag}", tag=f"mumu{tag}")
            nc.scalar.activation(mumu, mu_ps, AF.Square)
            vare = sm.tile([1, 512], f32, name=f"var{tag}", tag=f"var{tag}")
            nc.vector.scalar_tensor_tensor(out=vare, in0=m2_ps, scalar=EPS,
                                           in1=mumu, op0=ALU.add,
                                           op1=ALU.subtract)
            rstd = rsqrt_rows(vare, 1, tag)
            rstd_b = sm.tile([1, 512], bf16, name=f"rstdb{tag}",
                             tag=f"rstdb{tag}")
            nc.vector.tensor_copy(rstd_b, rstd)
            sh_b = sm.tile([1, 512], bf16, name=f"shb{tag}", tag=f"shb{tag}")
            nc.vector.scalar_tensor_tensor(out=sh_b, in0=mu_ps, scalar=-1.0,
                                           in1=rstd, op0=ALU.mult,
                                           op1=ALU.mult)
            return rstd_b, sh_b

        def ln_apply(xb, rstd_b, sh_b, tag):
            """out[fh] = xb[fh]*bcast(rstd) + bcast(sh), bf16."""
            arep = pms.tile([128, 512], f32, name="arep", tag="miscps")
            nc.tensor.matmul(arep, ones1, rstd_b, start=True, stop=True)
            brep = pms.tile([128, 512], f32, name="brep", tag="miscps")
            nc.tensor.matmul(brep, ones1, sh_b, start=True, stop=True)
            outs = []
            for fh in range(2):
                o = wk.tile([128, 512], bf16, name=f"ln{tag}{fh}",
                            tag=f"ln{tag}{fh}", bufs=1)
                nc.vector.tensor_mul(o, xb[fh], arep)
                nc.vector.tensor_add(o, o, brep)
                outs.append(o)
            return outs

        for ci in range(n_chunks):
            # ---------------- loads ----------------
            ht = ld.tile([128, 1024], f32, name="ht", tag="ht")
            hb = h_d[ci]
            nc.sync.dma_start(
                ht.rearrange("p (i f) -> p i f", i=4),
                bass.AP(tensor=hb.tensor, offset=hb.offset,
                        ap=[[256, 128], [128 * 256, 4], [1, 256]]))
            htb = ld.tile([128, 1024], bf16, name="htb", tag="htb")
            nc.vector.tensor_copy(htb, ht)
            hfmb = [wk.tile([128, 512], bf16, name=f"hfmb{fh}",
                            tag=f"hfmb{fh}") for fh in range(2)]
            for i in range(4):
                for fh in range(2):
                    tp = pms.tile([128, 128], bf16, name="tpi", tag="miscps")
                    nc.tensor.transpose(
                        tp, htb[:, i * 256 + fh * 128:i * 256 + (fh + 1) * 128],
                        ident)
                    nc.vector.tensor_copy(hfmb[fh][:, i * 128:(i + 1) * 128],
                                          tp)
            xyzp = sm.tile([3, 520], f32, name="xyzp", tag="xyzp")
            nc.vector.memset(xyzp, 0.0)
            nc.sync.dma_start(xyzp[:, 0:512],
                              xyz_d[ci].rearrange("t k -> k t"))
            vbase = val_d[ci]
            vr_u8 = sm.tile([4, 512], u8, name="vru", tag="vru")
            nc.sync.dma_start(
                vr_u8, bass.AP(tensor=vbase.tensor, offset=vbase.offset,
                               ap=[[0, 4], [1, 512]]))
            vrf = sm.tile([4, 512], bf16, name="vrf", tag="vrf")
            nc.vector.tensor_copy(vrf, vr_u8)
            vrs_u8 = sm.tile([4, 512], u8, name="vrsu", tag="vrsu")
            nc.vector.memset(vrs_u8, 0)
            for oi, off in enumerate(OFFSETS):
                nc.sync.dma_start(vrs_u8[oi:oi + 1, 0:C - off],
                                  val_d[ci, off:C][None, :])
            vrsf = sm.tile([4, 512], bf16, name="vrsf", tag="vrsf")
            nc.vector.tensor_copy(vrsf, vrs_u8)
            vcol_u8 = sm.tile([128, 4], u8, name="vcu", tag="vcu")
            nc.sync.dma_start(
                vcol_u8, bass.AP(tensor=vbase.tensor, offset=vbase.offset,
                                 ap=[[1, 128], [128, 4]]))
            vcolf = sm.tile([128, 4], f32, name="vcf", tag="vcf")
            nc.vector.tensor_copy(vcolf, vcol_u8)

            # ---------------- LN1 ----------------
            rstd1, sh1 = ln_rows(hfmb, "a")
            hin = ln_apply(hfmb, rstd1, sh1, "a")

            # ---------------- edge features ----------------
            delta = sm.tile([12, 512], bf16, name="delta", tag="delta")
            for oi, off in enumerate(OFFSETS):
                dlo = sm.tile([3, 512], bf16, name=f"dlo{oi}", tag=f"dlo{oi}")
                nc.vector.tensor_sub(dlo, xyzp[:, off:off + 512],
                                     xyzp[:, 0:512])
                nc.sync.dma_start(delta[3 * oi:3 * oi + 3], dlo)
            dsq = sm.tile([12, 512], bf16, name="dsq", tag="dsq")
            nc.gpsimd.tensor_mul(dsq, delta, delta)
            d2_ps = pms.tile([4, 512], f32, name="d2_ps", tag="miscps")
            nc.tensor.matmul(d2_ps, seldsq, dsq, start=True, stop=True)
            R = sm.tile([4, 512], f32, name="Rrows", tag="Rrows")
            nc.vector.tensor_scalar(out=R, in0=d2_ps, scalar1=1e-12,
                                    scalar2=None, op0=ALU.max)
            invd4 = rsqrt_rows(R, 4, "e")
            invd4b = sm.tile([4, 512], bf16, name="invd4b", tag="invd4b")
            nc.vector.tensor_copy(invd4b, invd4)
            dist4b = sm.tile([4, 512], bf16, name="dist4b", tag="dist4b")
            nc.vector.tensor_mul(dist4b, R, invd4)

            e20 = sm.tile([20, 512], bf16, name="e20", tag="e20")
            nc.sync.dma_start(e20[12:16], dist4b)
            m_all = sm.tile([4, 512], bf16, name="mall", tag="mall")
            nc.gpsimd.tensor_mul(m_all, vrf, vrsf)
            mm1 = sm.tile([4, 512], bf16, name="mm1", tag="mm1")
            nc.vector.tensor_scalar(out=mm1, in0=m_all, scalar1=1.0,
                                    scalar2=None, op0=ALU.subtract)
            nc.sync.dma_start(e20[16:20], mm1)
            invrep = pms.tile([12, 512], f32, name="invrep", tag="miscps")
            nc.tensor.matmul(invrep, selbc, invd4b, start=True, stop=True)
            nc.vector.tensor_mul(e20[0:12], delta, invrep)

            # ---------------- degree / agg init ----------------
            mrev = sm.tile([4, 512], bf16, name="mrev", tag="mrev")
            nc.vector.memset(mrev, 0.0)
            for oi, off in enumerate(OFFSETS):
                nc.sync.dma_start(mrev[oi:oi + 1, off:C],
                                  m_all[oi:oi + 1, 0:C - off])
            deg_ps = pms.tile([1, 512], f32, name="deg_ps", tag="miscps")
            nc.tensor.matmul(deg_ps, ones4, m_all, start=True, stop=False)
            nc.tensor.matmul(deg_ps, ones4, mrev, start=False, stop=True)
            deg_row = sm.tile([1, 512], bf16, name="degr", tag="degr")
            nc.vector.tensor_copy(deg_row, deg_ps)

            agg = [pagg.tile([128, 512], f32, name=f"agg{fh}", tag=f"agg{fh}")
                   for fh in range(2)]
            for fh in range(2):
                nc.tensor.matmul(agg[fh], b2r[:, fh * 128:(fh + 1) * 128],
                                 deg_row, start=True, stop=False,
                                 skip_group_check=True)

            # ---------------- P, Q ----------------
            P = wk.tile([128, 2048], bf16, name="P", tag="P")
            Q = wk.tile([128, 2048], bf16, name="Q", tag="Q")
            for m in range(4):
                pq = pmm.tile([128, 512], f32, name="mmps", tag="mmps")
                nc.tensor.matmul(pq, At[:, m * 128:(m + 1) * 128],
                                 hin2[:, 0:512], start=True, stop=False)
                nc.tensor.matmul(pq, At[:, 512 + m * 128:512 + (m + 1) * 128],
                                 hin2[:, 512:1024], start=False, stop=True)
                nc.vector.tensor_scalar(out=P[:, m * 512:(m + 1) * 512],
                                        in0=pq, scalar1=b1c[:, m:m + 1],
                                        scalar2=None, op0=ALU.add)
                pq2 = pmm.tile([128, 512], f32, name="mmps", tag="mmps")
                nc.tensor.matmul(pq2, Bt[:, m * 128:(m + 1) * 128],
                                 hin2[:, 0:512], start=True, stop=False)
                nc.tensor.matmul(pq2, Bt[:, 512 + m * 128:512 + (m + 1) * 128],
                                 hin2[:, 512:1024], start=False, stop=True)
                nc.vector.tensor_copy(Q[:, m * 512:(m + 1) * 512], pq2)

            # ---------------- messages ----------------
            for oi, off in enumerate(OFFSETS):
                N = C - off
                for d in range(2):  # 0=fwd (dst=i), 1=rev (dst=j)
                    po = 0 if d == 0 else off
                    qo = off if d == 0 else 0
                    tzg = wk.tile([128, 2048], bf16, name="tzg", tag="tzg")
                    for m in range(4):
                        blk = ((oi * 2 + d) * 4 + m) * 128
                        s = wk.tile([128, 512], bf16, name="spq",
                                    tag=f"spq{m % 2}", bufs=3)
                        eng = nc.gpsimd if m % 2 == 0 else nc.vector
                        eng.tensor_add(s[:, 0:N],
                                       P[:, m * 512 + po:m * 512 + po + N],
                                       Q[:, m * 512 + qo:m * 512 + qo + N])
                        ze = pz.tile([128, 512], f32, name="zps", tag="zps")
                        nc.tensor.matmul(ze[:, 0:N], CF[:, blk:blk + 128],
                                         e20[:, 0:N], start=True, stop=False)
                        nc.tensor.matmul(ze[:, 0:N], ident, s[:, 0:N],
                                         start=False, stop=True)
                        nc.scalar.activation(tzg[:, m * 512:m * 512 + N],
                                             ze[:, 0:N], AF.Gelu)
                    for fh in range(2):
                        dst = (agg[fh][:, 0:N] if d == 0
                               else agg[fh][:, off:off + N])
                        for k in range(4):
                            last = (oi == 3 and d == 1 and k == 3)
                            nc.tensor.matmul(
                                dst, W2[:, k * 256 + fh * 128:
                                        k * 256 + (fh + 1) * 128],
                                tzg[:, k * 512:k * 512 + N],
                                start=False, stop=last,
                                skip_group_check=True)

            # ---------------- update MLP ----------------
            h2b = [wk.tile([128, 512], bf16, name=f"h2{fh}", tag=f"h2{fh}",
                           bufs=1) for fh in range(2)]
            for fh in range(2):
                nc.vector.tensor_add(h2b[fh], hfmb[fh], agg[fh])
            rstd2, sh2 = ln_rows(h2b, "u")
            hun = ln_apply(h2b, rstd2, sh2, "u")
            gu = wk.tile([128, 4096], bf16, name="gu", tag="gu", bufs=1)
            for um in range(8):
                up = pmm.tile([128, 512], f32, name="mmps", tag="mmps")
                nc.tensor.matmul(up, U1a[:, um * 128:(um + 1) * 128], hun[0],
                                 start=True, stop=False)
                nc.tensor.matmul(up, U1b[:, um * 128:(um + 1) * 128], hun[1],
                                 start=False, stop=True)
                nc.scalar.activation(gu[:, um * 512:(um + 1) * 512],
                                     up, AF.Gelu, bias=bu1c[:, um:um + 1])
            of = [wk.tile([128, 512], bf16, name=f"of{fh}", tag=f"of{fh}",
                          bufs=1) for fh in range(2)]
            for fh in range(2):
                u2p = pmm.tile([128, 512], f32, name="mmps", tag="mmps")
                for k in range(8):
                    nc.tensor.matmul(
                        u2p, U2[:, k * 256 + fh * 128:k * 256 + (fh + 1) * 128],
                        gu[:, k * 512:(k + 1) * 512],
                        start=(k == 0), stop=(k == 7))
                nc.vector.scalar_tensor_tensor(
                    out=of[fh], in0=u2p, scalar=bu2c[:, fh:fh + 1],
                    in1=h2b[fh], op0=ALU.add, op1=ALU.add)

            # ---------------- transpose out + mask + store ----------------
            osb = ld.tile([128, 1024], f32, name="osb", tag="osb")
            for i in range(4):
                for fh in range(2):
                    tp = pms.tile([128, 128], bf16, name="tp", tag="miscps")
                    nc.tensor.transpose(tp, of[fh][:, i * 128:(i + 1) * 128],
                                        ident)
                    nc.scalar.activation(
                        osb[:, i * 256 + fh * 128:i * 256 + (fh + 1) * 128],
                        tp, AF.Copy, bias=0.0, scale=vcolf[:, i:i + 1])
            ob = out_d[ci]
            nc.sync.dma_start(
                bass.AP(tensor=ob.tensor, offset=ob.offset,
                        ap=[[256, 128], [128 * 256, 4], [1, 256]]),
                osb.rearrange("p (i f) -> p i f", i=4))

    return nc


def _get_nc(n_chunks):
    if n_chunks not in _nc_cache:
        _nc_cache[n_chunks] = _build(n_chunks)
    return _nc_cache[n_chunks]


# ---------------------------------------------------------------------------
# Host-side constant prep
# ---------------------------------------------------------------------------
def _prep_consts(w1, b1, w2, b2, ln_n_w, ln_n_b, u1, bu1, u2, bu2,
                 ln_u_w, ln_u_b):
    import ml_dtypes
    f = np.float32
    bf = ml_dtypes.bfloat16
    w1 = np.asarray(w1, f)
    u1 = np.asarray(u1, f)
    ln_n_w = np.asarray(ln_n_w, f)
    ln_n_b = np.asarray(ln_n_b, f)
    ln_u_w = np.asarray(ln_u_w, f)
    ln_u_b = np.asarray(ln_u_b, f)
    # Fold LN1 weight/bias into w1's h-halves / b1, LN2's into u1 / bu1.
    w1f = w1.copy()
    w1f[0:256] *= ln_n_w[:, None]
    w1f[256:512] *= ln_n_w[:, None]
    b1f = (np.asarray(b1, f) + ln_n_b @ w1[0:256] + ln_n_b @ w1[256:512])
    u1f = u1 * ln_u_w[:, None]
    bu1f = np.asarray(bu1, f) + ln_u_b @ u1

    A = np.concatenate([w1f[0:128, :], w1f[128:256, :]],
                       axis=1).astype(bf)
    B = np.concatenate([w1f[256:384, :], w1f[384:512, :]],
                       axis=1).astype(bf)
    W2 = np.zeros((128, 1024), f)
    for k in range(4):
        W2[:, k * 256:(k + 1) * 256] = w2[k * 128:(k + 1) * 128, :]
    U1 = np.concatenate([u1f[0:128, :], u1f[128:256, :]],
                        axis=1).astype(f8np)
    U2 = np.zeros((128, 2048), f)
    for k in range(8):
        U2[:, k * 256:(k + 1) * 256] = u2[k * 128:(k + 1) * 128, :]
    Cu = w1[512:515, :]          # [3, 512]
    Cd = w1[515, :]              # [512]
    CF = np.zeros((20, 4096), f)
    for oi in range(4):
        for d in range(2):
            sgn = 1.0 if d == 0 else -1.0
            for m in range(4):
                blk = ((oi * 2 + d) * 4 + m) * 128
                CF[3 * oi:3 * oi + 3, blk:blk + 128] = \
                    sgn * Cu[:, m * 128:(m + 1) * 128]
                CF[12 + oi, blk:blk + 128] = Cd[m * 128:(m + 1) * 128]
                CF[16 + oi, blk:blk + 128] = BIG
    return dict(
        A=A, B=B, W2=W2.astype(bf), U1=U1, U2=U2.astype(bf), CF=CF.astype(bf),
        b1c=np.ascontiguousarray(b1f.reshape(4, 128).T.astype(f)),
        bu1c=np.ascontiguousarray(bu1f.reshape(8, 128).T.astype(f)),
        bu2c=np.ascontiguousarray(np.asarray(bu2, f).reshape(2, 128).T),
        b2r=np.asarray(b2, f).reshape(1, 256).astype(bf),
        onesH=np.full((128, 1), 1.0 / H, bf),
        ones1=np.ones((1, 128), bf),
        ones4=np.ones((4, 1), bf),
        seldsq=_seldsq().astype(bf), selbc=_selbc().astype(bf),
        ident=np.eye(128, dtype=f).astype(bf),
        ident32=np.eye(128, dtype=f),
        seed=np.full((128, 24), MAGIC, np.int32),
    )


def _seldsq():
    s = np.zeros((12, 4), np.float32)
    for o in range(4):
        s[3 * o:3 * o + 3, o] = 1.0
    return s


def _selbc():
    s = np.zeros((4, 12), np.float32)
    for o in range(4):
        s[o, 3 * o:3 * o + 3] = 1.0
    return s


def _run(h, xyz, valid, consts, n_chunks_per_core, core_ids, trace=False):
    from concourse.bass_utils import run_bass_kernel_spmd

    _install_patch()
    nc = _get_nc(n_chunks_per_core)
    ncore = len(core_ids)
    in_maps = []
    for i in range(ncore):
        s = slice(i * n_chunks_per_core, (i + 1) * n_chunks_per_core)
        im = dict(consts)
        im["h"] = np.ascontiguousarray(h[s])
        im["xyz"] = np.ascontiguousarray(xyz[s])
        im["valid"] = np.ascontiguousarray(valid[s])
        in_maps.append(im)
    res = run_bass_kernel_spmd(nc, in_maps, core_ids=core_ids, trace=trace)
    outs = [res.results[i]["out"] for i in range(ncore)]
    return np.concatenate(outs, axis=0), res


def kernel(h, xyz, valid, ln_n_w, ln_n_b, w1, b1, w2, b2,
           ln_u_w, ln_u_b, u1, bu1, u2, bu2):
    h = np.asarray(h, np.float32)
    xyz = np.asarray(xyz, np.float32)
    valid = np.asarray(valid).astype(np.uint8)
    consts = _prep_consts(
        np.asarray(w1), np.asarray(b1), np.asarray(w2), np.asarray(b2),
        np.asarray(ln_n_w), np.asarray(ln_n_b), np.asarray(u1),
        np.asarray(bu1), np.asarray(u2), np.asarray(bu2),
        np.asarray(ln_u_w), np.asarray(ln_u_b))
    out, _ = _run(h, xyz, valid, consts, CPC, list(range(N_CORES)))
    return out.astype(np.float32)
